# revision 1
# baseline (speedup 1.0000x reference)
"""Trainium2 Bass kernel for group-quant (fake int8, V=64) + Linear.

reference math (per row of x):
    absmax over feature-groups of 64 -> delta = max(2*absmax/254, 1e-5)
    xq = clip(round(x/delta), -127, 127) * delta      (fake quant)
    out = xq @ W.T + b

Sharding: data-parallel on tokens across 8 cores (1024 rows each);
W (pre-transposed to [in,out] and cast fp16 on host) + b replicated.

Device pipeline per core (two token-halves for quant/matmul overlap):
  quant in natural layout (tokens on partitions): group absmax via
  free-dim reduce, exact RNE round via the +/-1.5*2^23 trick, dequant
  with fp16 output -> DRAM bounce -> XBAR DMA-transpose to get x~^T
  (contraction dim on partitions) -> fp16 matmuls (128x128x512) with
  fp32 PSUM accumulation over 32 k-tiles -> bias add -> out.
"""

import numpy as np

import concourse.bass as bass
import concourse.mybir as mybir
import concourse.tile as tile
from concourse.bass_utils import run_bass_kernel_spmd

N_CORES = 8
MAGIC = 1.5 * 2.0**23      # fp32 round-to-nearest-even constant
QSCALE = 1.0 / 127.0       # 2/(qmax-qmin) with qmax=127, qmin=-127
DELTA_MIN = 1e-5


def _split_multiwait(nc):
    """This walrus build allows at most ONE sync wait per instruction
    ("Too many sync wait commands", CoreV3GenImpl setupSyncWait) and none
    on Drain. Tile freely attaches several waits to one instruction, so
    post-process: move excess waits onto single-wait NoOps inserted just
    before the instruction on the same engine queue (semantics identical —
    the queue stalls at the nop instead of at the instruction)."""
    nid = 0
    for fn in nc.m.functions:
        for bb in fn.blocks:
            insts = list(bb.instructions)
            out = []
            changed = False
            for inst in insts:
                si = inst.sync_info
                waits = list(si.on_wait) if si is not None and si.on_wait else []
                limit = 0 if type(inst).__name__ == "InstDrain" else 1
                if len(waits) > limit:
                    changed = True
                    keep = waits[len(waits) - limit :] if limit else []
                    for w in waits[: len(waits) - limit]:
                        nid += 1
                        out.append(
                            mybir.InstNoOp(
                                name=f"WSPLIT-{nid}",
                                engine=inst.engine,
                                bass_nofuse=True,
                                ins=[],
                                outs=[],
                                sync_info=mybir.SyncInfo(on_wait=[w], on_update=[]),
                            )
                        )
                    si.on_wait = keep
                out.append(inst)
            if changed:
                try:
                    bb.instructions = out
                except Exception:
                    bb.instructions[:] = out


def build(T=1024, K=4096, O=4096, V=64, chunks=(4, 4), OC=512, split=True,
          wpack=True, sbuf_tr=False, wbufs=8, skip_quant=False, skip_tr=False,
          per_t_tr=False):
    f32, f16 = mybir.dt.float32, mybir.dt.float16
    P = 128
    G = K // V                 # quant groups per row
    KT = K // P                # contraction tiles
    NOC = O // OC              # output chunks
    QW = 4                     # W-load quarters per o-chunk
    KQ = KT // QW
    assert sum(chunks) * P == T

    nc = bass.Bass()
    x = nc.dram_tensor("x", [T, K], f32, kind="ExternalInput")
    if wpack:
        wt = nc.dram_tensor("wt", [NOC, P, KT * OC], f16, kind="ExternalInput")
    else:
        wt = nc.dram_tensor("wt", [K, O], f16, kind="ExternalInput")
    bvec = nc.dram_tensor("b", [O], f32, kind="ExternalInput")
    out = nc.dram_tensor("out", [T, O], f32, kind="ExternalOutput")
    xtd = nc.dram_tensor("xtd", [K, T], f16, kind="ExternalInput") if skip_tr else None
    if skip_quant:
        xh = nc.dram_tensor("xh", [T, K], f16, kind="ExternalInput")
    else:
        xh = nc.dram_tensor("xh", [T, K], f16)  # internal bounce for x~

    wt3 = None if wpack else wt.rearrange("(kt p) o -> p kt o", p=P)
    mult = mybir.AluOpType.mult
    add = mybir.AluOpType.add
    sub = mybir.AluOpType.subtract
    amax_op = mybir.AluOpType.max

    with tile.TileContext(nc) as tc:
        with (
            tc.tile_pool(name="xq", bufs=2) as pool_x,
            tc.tile_pool(name="xh", bufs=2) as pool_xh,
            tc.tile_pool(name="st", bufs=4) as pool_s,
            tc.tile_pool(name="xt", bufs=1) as pool_xt,
            tc.tile_pool(name="w", bufs=wbufs) as pool_w,
            tc.tile_pool(name="bias", bufs=3) as pool_b,
            tc.tile_pool(name="o", bufs=8) as pool_o,
            tc.tile_pool(name="ps", bufs=8, space="PSUM") as pool_ps,
        ):
            row0 = 0
            for h, TT in enumerate(chunks):
                TH = TT * P
                xT_cur = (
                    pool_xt.tile([P, KT, TH], f16, tag=f"xT{h % 2}", name=f"xTc{h}")
                    if sbuf_tr else None
                )
                xT_pre = (
                    pool_xt.tile([P, KT, TH], f16, tag=f"xT{h % 2}", name=f"xTp{h}")
                    if per_t_tr else None
                )
                # ---- group fake-quant, natural layout ----
                for t in range(TT) if not skip_quant else []:
                    r0 = row0 + t * P
                    xt_ = pool_x.tile([P, K], f32, tag="xq")
                    nc.gpsimd.dma_start(out=xt_[:], in_=x[r0 : r0 + P, :])
                    x3 = xt_.rearrange("p (g v) -> p g v", v=V)
                    amax = pool_s.tile([P, G], f32, tag="amax")
                    nc.vector.tensor_reduce(
                        out=amax[:], in_=x3, axis=mybir.AxisListType.X,
                        op=amax_op, apply_absolute_value=True,
                    )
                    delta = pool_s.tile([P, G], f32, tag="delta")
                    nc.vector.tensor_scalar(
                        out=delta[:], in0=amax[:],
                        scalar1=QSCALE, scalar2=DELTA_MIN, op0=mult, op1=amax_op,
                    )
                    recip = pool_s.tile([P, G], f32, tag="recip")
                    nc.vector.reciprocal(out=recip[:], in_=delta[:])
                    # x / delta  (broadcast recip over each group of V)
                    nc.vector.tensor_tensor(
                        out=x3, in0=x3,
                        in1=recip[:, :, None].to_broadcast((P, G, V)), op=mult,
                    )
                    # exact fp32 round-to-nearest-even; |x/delta| <= 127 < 2^22
                    nc.vector.tensor_scalar(
                        out=xt_[:], in0=xt_[:],
                        scalar1=MAGIC, scalar2=MAGIC, op0=add, op1=sub,
                    )
                    # dequant, cast to fp16 (integers <=127 are exact in fp16)
                    xh_t = pool_xh.tile([P, K], f16, tag="xh")
                    nc.vector.tensor_tensor(
                        out=xh_t.rearrange("p (g v) -> p g v", v=V), in0=x3,
                        in1=delta[:, :, None].to_broadcast((P, G, V)), op=mult,
                    )
                    if sbuf_tr:
                        for k in range(KT):
                            eng = nc.scalar if k % 2 == 0 else nc.sync
                            eng.dma_start_transpose(
                                xT_cur[:, k, t * P : (t + 1) * P],
                                xh_t[:, k * P : (k + 1) * P],
                            )
                    else:
                        nc.gpsimd.dma_start(out=xh[r0 : r0 + P, :], in_=xh_t[:])
                        if per_t_tr:
                            for k in range(KT):
                                nc.scalar.dma_start_transpose(
                                    xT_pre[:, k, t * P : (t + 1) * P],
                                    xh[r0 : r0 + P, k * P : (k + 1) * P],
                                )

                # ---- transpose x~ -> x~^T tiles (both HWDGE queues) ----
                if sbuf_tr:
                    xT = xT_cur
                elif per_t_tr:
                    xT = xT_pre
                else:
                    xT = pool_xt.tile([P, KT, TH], f16, tag=f"xT{h % 2}", name=f"xTn{h}")
                if skip_tr:
                    nc.scalar.dma_start(
                        out=xT[:],
                        in_=xtd.rearrange("(kt p) t -> p kt t", p=P)[
                            :, :, row0 : row0 + TH
                        ],
                    )
                elif not sbuf_tr and not per_t_tr:
                    # all transposes on the scalar HWDGE queue, isolated from
                    # copy-mode DMAs (xbar_mode transitions corrupt/hang)
                    for k in range(KT):
                        nc.scalar.dma_start_transpose(
                            xT[:, k, :], xh[row0 : row0 + TH, k * P : (k + 1) * P]
                        )

                # ---- matmul + bias ----
                for oc in range(NOC):
                    wq = []
                    for q in range(QW):
                        wqt = pool_w.tile([P, KQ, OC], f16, tag="w", name=f"w{h}_{oc}_{q}")
                        if wpack:
                            nc.sync.dma_start(
                                out=wqt.rearrange("p kq o -> p (kq o)"),
                                in_=wt[oc][:, q * KQ * OC : (q + 1) * KQ * OC],
                            )
                        else:
                            nc.sync.dma_start(
                                out=wqt[:],
                                in_=wt3[:, q * KQ : (q + 1) * KQ, oc * OC : (oc + 1) * OC],
                            )
                        wq.append(wqt)
                    btile = pool_b.tile([P, OC], f32, tag="bias")
                    bsl = bvec[oc * OC : (oc + 1) * OC]
                    b_bcast = bass.AP(
                        tensor=bsl.tensor, offset=bsl.offset, ap=[[0, P], *bsl.ap]
                    )
                    nc.sync.dma_start(out=btile[:], in_=b_bcast)
                    for t in range(TT):
                        ps = pool_ps.tile([P, OC], f32, tag="ps")
                        for k in range(KT):
                            nc.tensor.matmul(
                                ps[:],
                                xT[:, k, t * P : (t + 1) * P],
                                wq[k // KQ][:, k % KQ, :],
                                start=(k == 0),
                                stop=(k == KT - 1),
                            )
                        ot = pool_o.tile([P, OC], f32, tag="o")
                        nc.vector.tensor_tensor(out=ot[:], in0=ps[:], in1=btile[:], op=add)
                        r0 = row0 + t * P
                        nc.sync.dma_start(
                            out=out[r0 : r0 + P, oc * OC : (oc + 1) * OC], in_=ot[:]
                        )
                row0 += TH
    if split:
        _split_multiwait(nc)
    return nc


_CACHED = {}

# test-harness knobs (kernel() defaults are what the grader uses)
TRACE = False
LAST_RESULT = None


def _get_nc(shape_key):
    if shape_key not in _CACHED:
        T, K, O = shape_key
        _CACHED[shape_key] = build(T=T, K=K, O=O)
    return _CACHED[shape_key]


def pack_w(W: np.ndarray, OC: int = 512, P: int = 128) -> np.ndarray:
    # [out,in] -> W^T [in,out] fp16, packed [NOC, P, KT*OC] so each per-core
    # o-chunk W load is one fully contiguous DMA
    K, O = W.shape[1], W.shape[0]
    KT, NOC = K // P, O // OC
    wt = np.ascontiguousarray(W.T).astype(np.float16)         # [K, O]
    z = wt.reshape(KT, P, NOC, OC).transpose(2, 1, 0, 3)      # [NOC, P, KT, OC]
    return np.ascontiguousarray(z.reshape(NOC, P, KT * OC))


def kernel(x: np.ndarray, W: np.ndarray, b: np.ndarray) -> np.ndarray:
    global LAST_RESULT
    n, k = x.shape               # 8192, 4096
    o = W.shape[0]               # 4096
    assert n % N_CORES == 0
    tpc = n // N_CORES
    nc = _get_nc((tpc, k, o))

    wt = pack_w(W)
    b32 = np.ascontiguousarray(b.astype(np.float32))
    xs = np.ascontiguousarray(x.astype(np.float32)).reshape(N_CORES, tpc, k)
    in_maps = [{"x": xs[i], "wt": wt, "b": b32} for i in range(N_CORES)]
    res = run_bass_kernel_spmd(nc, in_maps, list(range(N_CORES)), trace=TRACE)
    LAST_RESULT = res
    return np.concatenate([res.results[i]["out"] for i in range(N_CORES)], axis=0)

